# revision 33
# baseline (speedup 1.0000x reference)
import sys

sys.path.insert(0, "/opt/trn_rl_repo")

import numpy as np
import ml_dtypes

BF16 = ml_dtypes.bfloat16

# Problem constants (nn_Arch7V2Layer: F=131072 flat nodes in 4096 subgraphs of 32,
# N=32768 canonical nodes, H=128, 524288 edges in each graph)
H = 128
P = 128
S_ = 4096
K_ = 32
F_ = S_ * K_
N_TOTAL = 32768
M = 8                    # cores
FC = F_ // M             # 16384 flat rows per core
NC_SH = N_TOTAL // M     # 4096 canonical rows per core
NT_C = NC_SH // P        # 32 canonical tiles per core
NT_F = FC // P           # 128 flat tiles per core
XS_T = N_TOTAL // P      # 256 canonical tiles (full range, x_sum partial)
BN_EPS = 1e-5


def _wrap_idx16(block):
    """dma_gather index layout: idx[p, s] = flat[s*16 + (p%16)], 128 partitions."""
    n = block.shape[0]
    assert n % 16 == 0
    m = block.reshape(n // 16, 16).T.astype(np.int16)   # [16, n/16]
    return np.tile(m, (8, 1))                           # [128, n/16]


def _host_prep(h_flat, intra_ei, valid, node_ids, edge_index, sub_batch,
               root_flat_idx, is_root):
    """Index-only preprocessing + sharding. Returns (meta, per-core input dicts)."""
    h_flat = np.asarray(h_flat, np.float32)
    intra_ei = np.asarray(intra_ei, np.int64)
    valid = np.asarray(valid)
    node_ids = np.asarray(node_ids, np.int64)
    edge_index = np.asarray(edge_index, np.int64)
    sub_batch = np.asarray(sub_batch, np.int64)
    root_flat_idx = np.asarray(root_flat_idx, np.int64)
    is_root = np.asarray(is_root)

    # ---- local graph degrees (self loops included) over the full flat graph
    ldeg = np.ones(F_, np.float64)
    np.add.at(ldeg, intra_ei[1], 1.0)
    ldinv = ldeg ** -0.5

    # ---- global graph degrees
    gdeg = np.ones(N_TOTAL, np.float64)
    np.add.at(gdeg, edge_index[1], 1.0)
    gdinv = gdeg ** -0.5

    # ---- global scatter-mean counts
    vmask = node_ids >= 0
    cids = np.where(vmask, node_ids, 0)
    cnt = np.zeros(N_TOTAL, np.float64)
    np.add.at(cnt, cids[vmask], 1.0)
    wxs_all = 1.0 / np.maximum(cnt, 1.0)

    root_ids = node_ids[root_flat_idx]          # [4096]
    rvalid = root_ids >= 0
    crids = np.where(rvalid, root_ids, 0)
    cntv = np.zeros(N_TOTAL, np.float64)
    np.add.at(cntv, crids[rvalid], 1.0)
    wvv_all = 1.0 / np.maximum(cntv, 1.0)

    # global edges + self loops, with weights
    ge_src = np.concatenate([edge_index[0], np.arange(N_TOTAL, dtype=np.int64)])
    ge_dst = np.concatenate([edge_index[1], np.arange(N_TOTAL, dtype=np.int64)])
    ge_w = gdinv[ge_src] * gdinv[ge_dst]

    root_of_flat = root_flat_idx[sub_batch]     # [F]

    # ---- uniform padding budgets (must be identical across cores)
    # K_G: global-edge chunks per canonical tile
    owner = ge_dst // NC_SH
    tile_g = ge_dst // P
    cnt_gt = np.bincount(tile_g, minlength=XS_T)
    K_G = int(np.ceil(cnt_gt.max() / P))
    # K_XS: x_sum rows per canonical tile per core
    maxr = 0
    for c in range(M):
        ids_c = cids[c * FC:(c + 1) * FC]
        vm_c = vmask[c * FC:(c + 1) * FC]
        bc = np.bincount(ids_c[vm_c] // P, minlength=XS_T)
        maxr = max(maxr, int(bc.max()))
    K_XS = int(np.ceil(maxr / P))
    assert K_XS == 1, f"K_XS={K_XS} unsupported"
    # K_VV: root rows per canonical tile
    bcv = np.bincount(crids[rvalid] // P, minlength=XS_T)
    K_VV = int(np.ceil(bcv.max() / P))
    assert K_VV == 1, f"K_VV={K_VV} unsupported"

    if K_G % 2 == 1:
        K_G += 1                                # keep 4096-idx gather blocks exact
    NG_E = (NT_C * K_G * P) // 4096             # dma_gather groups (32 chunks each)
    assert (NT_C * K_G * P) % 4096 == 0

    meta = dict(K_G=K_G, NG_E=NG_E)
    cores = []
    for c in range(M):
        lo = c * FC
        rows = slice(lo, lo + FC)
        h32 = np.ascontiguousarray(h_flat[rows])

        # ---- local block matrices A^T (adjacency incl. self loops) and B^T (root broadcast)
        es, ed = intra_ei[0], intra_ei[1]
        m = (ed >= lo) & (ed < lo + FC)
        assert ((es[m] >= lo) & (es[m] < lo + FC)).all(), "intra edge crosses shard"
        sl = (es[m] - lo).astype(np.int64)
        dl = (ed[m] - lo).astype(np.int64)
        AT = np.zeros((NT_F, P, P), np.float32)
        w_l = (ldinv[es[m]] * ldinv[ed[m]]).astype(np.float64)
        g_of = dl // P
        assert (sl // P == g_of).all()
        np.add.at(AT, (g_of, sl % P, dl % P), w_l)
        loc = np.arange(FC)
        AT[loc // P, loc % P, loc % P] += (ldinv[lo:lo + FC] ** 2)
        BT = np.zeros((NT_F, P, P), np.float32)
        rof = root_of_flat[rows] - lo
        assert ((rof >= 0) & (rof < FC)).all(), "root outside shard"
        assert (rof // P == loc // P).all()
        BT[loc // P, rof % P, loc % P] = 1.0

        isrootF = np.ascontiguousarray(
            is_root[rows].astype(np.float32).reshape(NT_F, P).T)
        validF = np.ascontiguousarray(
            valid[rows].astype(np.float32).reshape(NT_F, P).T)

        # ---- x_sum partial: sorted-by-id rows, padded per canonical tile
        ids_c = cids[rows]
        vm_c = vmask[rows]
        order = np.argsort(ids_c + (~vm_c) * (1 << 40), kind="stable")
        order = order[vm_c[order]]
        sids = ids_c[order]
        hperm = np.zeros((XS_T * K_XS * P, H), np.float32)
        Sxs = np.zeros((XS_T * K_XS, P, P), np.float32)
        tiles = sids // P
        starts = np.searchsorted(tiles, np.arange(XS_T))
        ends = np.searchsorted(tiles, np.arange(XS_T), side="right")
        for t in range(XS_T):
            n = ends[t] - starts[t]
            if n == 0:
                continue
            rsel = order[starts[t]:ends[t]]
            base = t * K_XS * P
            hperm[base:base + n] = h_flat[rows][rsel]
            Sxs[t * K_XS + (np.arange(n) // P), np.arange(n) % P,
                sids[starts[t]:ends[t]] % P] = wxs_all[sids[starts[t]:ends[t]]]

        # ---- global edges owned by this core, sorted by dst, padded per tile
        selg = owner == c
        gs = ge_src[selg]
        gd = ge_dst[selg]
        gw = ge_w[selg]
        og = np.argsort(gd, kind="stable")
        gs, gd, gw = gs[og], gd[og], gw[og]
        tl = (gd - c * NC_SH) // P
        st = np.searchsorted(tl, np.arange(NT_C))
        en = np.searchsorted(tl, np.arange(NT_C), side="right")
        esrc_pad = np.zeros(NT_C * K_G * P, np.int64)
        Sg = np.zeros((NT_C * K_G, P, P), np.float32)
        for t in range(NT_C):
            n = en[t] - st[t]
            assert n <= K_G * P
            base = t * K_G * P
            esrc_pad[base:base + n] = gs[st[t]:en[t]]
            Sg[t * K_G + (np.arange(n) // P), np.arange(n) % P,
               gd[st[t]:en[t]] % P] = gw[st[t]:en[t]]
        ep = esrc_pad
        esrc_phys = (ep // 512) * 512 + (ep % P) * 4 + (ep // P) % 4
        eidx = np.concatenate(
            [_wrap_idx16(esrc_phys[g * 4096:(g + 1) * 4096]) for g in range(NG_E)],
            axis=1)

        # ---- flat gathers (node_ids per row), root gathers
        fidx = np.concatenate(
            [_wrap_idx16(cids[rows][g * 2048:(g + 1) * 2048]) for g in range(8)],
            axis=1)
        rsel_roots = (root_flat_idx >= lo) & (root_flat_idx < lo + FC)
        r_glob = np.nonzero(rsel_roots)[0]
        assert len(r_glob) == S_ // M, f"roots per core {len(r_glob)}"
        my_root_ids = crids[r_glob]
        ridx = _wrap_idx16(my_root_ids)                       # [128, 32]
        rloc = _wrap_idx16((root_flat_idx[r_glob] - lo))      # [128, 32] int16

        # P matrices: roots of each output tile (4 per tile, in order)
        Pm = np.zeros((4, NT_F, P), np.float32)
        rl_flat = (root_flat_idx[r_glob] - lo)
        tg = rl_flat // P
        assert (np.bincount(tg, minlength=NT_F) == 4).all(), "need 4 roots/tile"
        kk_idx = np.zeros(NT_F, np.int64)
        for j, (t, rr) in enumerate(zip(tg, rl_flat % P)):
            Pm[kk_idx[t], t, rr] = 1.0
            kk_idx[t] += 1

        # ---- x_vv: roots landing in this core's canonical shard
        selv = rvalid & (crids // NC_SH == c)
        rv = np.nonzero(selv)[0]
        rvids = crids[rv]
        ov = np.argsort(rvids, kind="stable")
        rv, rvids = rv[ov], rvids[ov]
        tlv = (rvids - c * NC_SH) // P
        stv = np.searchsorted(tlv, np.arange(NT_C))
        env = np.searchsorted(tlv, np.arange(NT_C), side="right")
        vidx = np.zeros((NT_C * K_VV * P,), np.int32)
        Svv = np.zeros((NT_C * K_VV, P, P), np.float32)
        for t in range(NT_C):
            n = env[t] - stv[t]
            assert n <= K_VV * P
            base = t * K_VV * P
            vidx[base:base + n] = rv[stv[t]:env[t]]
            Svv[t * K_VV + (np.arange(n) // P), np.arange(n) % P,
                rvids[stv[t]:env[t]] % P] = wvv_all[rvids[stv[t]:env[t]]]
        vidx = _wrap_idx16(vidx)                              # [128, 256] int16

        def tile4(a, nb):
            return np.ascontiguousarray(
                a.reshape(nb, 4, P, a.shape[-1]).transpose(0, 2, 1, 3))

        cores.append(dict(
            h=h32,
            h_t=tile4(h32, NT_F // 4).astype(np.float32),
            hperm=hperm.astype(BF16),
            hperm_t=tile4(hperm, XS_T // 4).astype(BF16),
            AT=AT.astype(BF16), BT=BT.astype(BF16),
            AT_t=tile4(AT.reshape(-1, P), NT_F // 4).astype(BF16),
            BT_t=tile4(BT.reshape(-1, P), NT_F // 4).astype(BF16),
            Sxs=Sxs.astype(BF16), Sg=Sg.astype(BF16), Svv=Svv.astype(BF16),
            Sxs_t=tile4(Sxs.reshape(-1, P), XS_T // 4).astype(BF16),
            Sg_t=np.ascontiguousarray(
                Sg.reshape(NG_E, 32, P, P).transpose(0, 2, 1, 3)).astype(BF16),
            Svv_t=tile4(Svv.reshape(-1, P), NT_C // 4).astype(BF16),
            eidx=eidx.astype(np.int16), fidx=fidx.astype(np.int16),
            vidx=vidx.astype(np.int16), rloc=rloc.astype(np.int16),
            isrootF=isrootF, validF=validF,
        ))
    return meta, cores


def _host_weights(ins):
    """Per-core replicated weight/vector uploads."""
    w = {}
    for k in ("skip_W", "vv_W", "kk_W", "lc_W", "lcr_W", "gc_W", "gcr_W"):
        w[k] = np.asarray(ins[k], np.float32)
    for k in ("lc_W", "lcr_W", "gc_W", "gcr_W"):
        w[k + "T"] = np.ascontiguousarray(w[k].T)
    vec_names = ("skip_b", "kk_b", "vv_b", "lbn_g", "lbn_b", "lbnr_g", "lbnr_b",
                 "gbn_g", "gbn_b", "gbnr_g", "gbnr_b")
    w["vecs"] = np.stack([np.asarray(ins[k], np.float32) for k in vec_names],
                         axis=1)  # [128, 11]
    return w


class _StopBuild(Exception):
    pass


def _build_nc(K_G, NG_E, stage=99):
    from concourse import bass, bacc, mybir
    import concourse.tile as tile
    from concourse.masks import make_identity

    f32 = mybir.dt.float32
    bf16 = mybir.dt.bfloat16
    AF = mybir.ActivationFunctionType
    ALU = mybir.AluOpType

    nc = bacc.Bacc(None, num_swdge_queues=4)
    dp = nc.declare_dram_parameter
    E_h = dp("h", [FC, H], f32, isOutput=False)
    E_ht = dp("h_t", [NT_F // 4, P, 4, H], f32, isOutput=False)
    E_hperm = dp("hperm_t", [XS_T // 4, P, 4, H], bf16, isOutput=False)
    E_AT = dp("AT_t", [NT_F // 4, P, 4, P], bf16, isOutput=False)
    E_BT = dp("BT_t", [NT_F // 4, P, 4, P], bf16, isOutput=False)
    E_Sxs = dp("Sxs_t", [XS_T // 4, P, 4, P], bf16, isOutput=False)
    E_Sg = dp("Sg_t", [NG_E, P, 32, P], bf16, isOutput=False)
    E_Svv = dp("Svv_t", [NT_C // 4, P, 4, P], bf16, isOutput=False)
    E_eidx = dp("eidx", [P, NG_E * 256], mybir.dt.int16, isOutput=False)
    E_fidx = dp("fidx", [P, 8 * 128], mybir.dt.int16, isOutput=False)
    E_vidx = dp("vidx", [P, 256], mybir.dt.int16, isOutput=False)
    E_rloc = dp("rloc", [P, 32], mybir.dt.int16, isOutput=False)
    E_isroot = dp("isrootF", [P, NT_F], f32, isOutput=False)
    E_valid = dp("validF", [P, NT_F], f32, isOutput=False)
    WN = ("skip_W", "vv_W", "kk_W", "lc_W", "lcr_W", "gc_W", "gcr_W",
          "lc_WT", "lcr_WT", "gc_WT", "gcr_WT")
    E_W = {k: dp(k, [P, P], f32, isOutput=False) for k in WN}
    E_vecs = dp("vecs", [P, 11], f32, isOutput=False)
    E_out = dp("out", [FC, H], f32, isOutput=True)

    # collective bounce buffers
    ag1_in = nc.dram_tensor("ag1_in", [S_ // M, H], bf16)
    ag1_out = nc.dram_tensor("ag1_out", [S_, H], bf16, addr_space="Shared")
    rs_in = nc.dram_tensor("rs_in", [XS_T // 4, P, 4, H], bf16)
    rs_out = nc.dram_tensor("rs_out", [XS_T // 4 // M, P, 4, H], bf16)
    ag2_out = nc.dram_tensor("ag2_out", [N_TOTAL, H], bf16, addr_space="Shared")
    ag3_in = nc.dram_tensor("ag3_in", [NC_SH, 2 * H], bf16)
    ag3_out = nc.dram_tensor("ag3_out", [N_TOTAL, 2 * H], bf16, addr_space="Shared")
    ar_in = nc.dram_tensor("ar_in", [P, 8], f32)
    ar_out = nc.dram_tensor("ar_out", [P, 8], f32, addr_space="Shared")
    RG = [list(range(M))]

    with tile.TileContext(nc) as tc:
        ctx_pools = []

        def pool(name, bufs, space="SBUF"):
            p_ = tc.tile_pool(name=name, bufs=bufs, space=space)
            ctx_pools.append(p_)
            return p_.__enter__()

        try:
            const = pool("const", 1)
            io = pool("io", 2)
            ps = pool("ps", 1, "PSUM")
            big = pool("big", 1)
            stat = pool("stat", 1)
            gat = pool("gat", 2)
            sgp = pool("sgp", 3)

            def sq(name):
                return ps.tile([P, P], f32, name=name, tag="sq", bufs=4, space="PSUM")

            def wide(name):
                return ps.tile([P, 512], f32, name=name, tag="wide", bufs=2,
                               space="PSUM")

            def tp(name):
                return ps.tile([P, P], bf16, name=name, tag="tp", bufs=2, space="PSUM")

            # ---------- constants ----------
            ident_f = const.tile([P, P], f32)
            make_identity(nc, ident_f[:])
            eps_t = const.tile([P, 1], f32)
            nc.vector.memset(eps_t[:], BN_EPS)
            ident_b = const.tile([P, P], bf16)
            nc.vector.tensor_copy(out=ident_b[:], in_=ident_f[:])

            Wsb = {}
            for k in WN:
                t_f = io.tile([P, P], f32, name=f"wld_{k}")
                nc.sync.dma_start(out=t_f[:], in_=E_W[k][:])
                if k.endswith("T"):
                    Wsb[k] = const.tile([P, P], f32, name=f"w_{k}")
                    nc.vector.tensor_copy(out=Wsb[k][:], in_=t_f[:])
                else:
                    Wsb[k] = const.tile([P, P], bf16, name=f"w_{k}")
                    nc.vector.tensor_copy(out=Wsb[k][:], in_=t_f[:])
            vecs = const.tile([P, 11], f32)
            nc.sync.dma_start(out=vecs[:], in_=E_vecs[:])
            isroot_sb = const.tile([P, NT_F], f32)
            nc.sync.dma_start(out=isroot_sb[:], in_=E_isroot[:])
            valid_sb = const.tile([P, NT_F], f32)
            nc.sync.dma_start(out=valid_sb[:], in_=E_valid[:])
            eidx_sb = const.tile([P, NG_E * 256], mybir.dt.int16)
            nc.sync.dma_start(out=eidx_sb[:], in_=E_eidx[:])
            fidx_sb = const.tile([P, 8 * 128], mybir.dt.int16)
            nc.sync.dma_start(out=fidx_sb[:], in_=E_fidx[:])
            vidx_sb = const.tile([P, 256], mybir.dt.int16)
            nc.sync.dma_start(out=vidx_sb[:], in_=E_vidx[:])
            rloc_sb = const.tile([P, 32], mybir.dt.int16)
            nc.sync.dma_start(out=rloc_sb[:], in_=E_rloc[:])

            # ---------- 1. extract local roots, cast bf16, AG1 ----------
            ag1_stage = io.tile([P, 4, H], bf16, name="ag1_stage")
            rg = io.tile([P, 4, H], f32, name="rootg")
            nc.gpsimd.dma_gather(
                rg[:], E_h[:], rloc_sb[:], num_idxs=512, num_idxs_reg=512,
                elem_size=H, single_packet=False, queue_num=3)
            nc.vector.tensor_copy(out=ag1_stage[:], in_=rg[:])
            nc.sync.dma_start(
                out=ag1_in[:].rearrange("(a p) f -> p a f", p=P), in_=ag1_stage[:])
            nc.gpsimd.collective_compute(
                "AllGather", ALU.bypass, replica_groups=RG,
                ins=[ag1_in[:].opt()], outs=[ag1_out[:].opt()])

            # ---------- 2. x_sum partial via S matmuls, then RS ----------
            for tb in range(XS_T // 8):
                hp8 = io.tile([P, 2, 4, H], bf16, name="hp8")
                nc.scalar.dma_start(
                    out=hp8[:],
                    in_=E_hperm[tb * 2:(tb + 1) * 2].rearrange("b p a f -> p b a f"))
                sx8 = io.tile([P, 2, 4, P], bf16, name="sx8")
                nc.scalar.dma_start(
                    out=sx8[:],
                    in_=E_Sxs[tb * 2:(tb + 1) * 2].rearrange("b p a f -> p b a f"))
                st8 = io.tile([P, 2, 4, H], bf16, name="xs_st")
                for cc in range(8):
                    pxs = sq("pxs")
                    nc.tensor.matmul(out=pxs[:], lhsT=sx8[:, cc // 4, cc % 4, :],
                                     rhs=hp8[:, cc // 4, cc % 4, :],
                                     start=True, stop=True)
                    if cc % 2 == 0:
                        nc.vector.tensor_copy(out=st8[:, cc // 4, cc % 4, :], in_=pxs[:])
                    else:
                        nc.scalar.copy(out=st8[:, cc // 4, cc % 4, :], in_=pxs[:])
                nc.sync.dma_start(
                    out=rs_in[tb * 2:(tb + 1) * 2].rearrange("b p a f -> p b a f"),
                    in_=st8[:])
            nc.gpsimd.collective_compute(
                "ReduceScatter", ALU.add, replica_groups=RG,
                ins=[rs_in[:].opt()], outs=[rs_out[:].opt()])

            nc.gpsimd.collective_compute(
                "AllGather", ALU.bypass, replica_groups=RG,
                ins=[rs_out[:].opt()], outs=[ag2_out[:].opt()])

            # ---------- 3. local phase-1: Z^T (feat-major, bf16) ----------
            if stage < 2:
                raise _StopBuild()
            ZT = big.tile([P, FC], bf16)
            for gb in range(NT_F // 4):
                h4 = io.tile([P, 4, H], f32, name="h4")
                nc.sync.dma_start(out=h4[:], in_=E_ht[gb])
                a4 = io.tile([P, 4, P], bf16, name="a4")
                nc.sync.dma_start(out=a4[:], in_=E_AT[gb])
                for cc in range(4):
                    g = gb * 4 + cc
                    hbf = io.tile([P, H], bf16, name="hbf")
                    nc.vector.tensor_copy(out=hbf[:], in_=h4[:, cc, :])
                    pz = sq("pz")
                    nc.tensor.matmul(out=pz[:], lhsT=hbf[:], rhs=a4[:, cc, :],
                                     start=True, stop=True)
                    sl = slice(g * P, (g + 1) * P)
                    nc.vector.tensor_copy(out=ZT[:, sl], in_=pz[:])

            # ---------- 4. local BN stats (pre-AR partial sums) ----------
            l_sum_nr = stat.tile([P, NT_F // 4], f32)
            l_sq_nr = stat.tile([P, NT_F // 4], f32)
            l_sum_r = stat.tile([P, NT_F // 4], f32)
            l_sq_r = stat.tile([P, NT_F // 4], f32)
            scr = io.tile([P, 512], f32, name="scr")
            for cg in range(NT_F // 4):
                sl = slice(cg * 512, (cg + 1) * 512)
                py = wide("py")
                nc.tensor.matmul(out=py[:], lhsT=Wsb["lc_W"][:], rhs=ZT[:, sl],
                                 start=True, stop=True)
                nc.scalar.activation(out=scr[:], in_=py[:], func=AF.Identity,
                                     accum_out=l_sum_nr[:, cg:cg + 1])
                nc.scalar.activation(out=scr[:], in_=py[:], func=AF.Square,
                                     accum_out=l_sq_nr[:, cg:cg + 1])
                py2 = wide("py2")
                nc.tensor.matmul(out=py2[:], lhsT=Wsb["lcr_W"][:], rhs=ZT[:, sl],
                                 start=True, stop=True)
                nc.vector.tensor_reduce(out=l_sum_r[:, cg:cg + 1], in_=py2[:],
                                        axis=mybir.AxisListType.X, op=ALU.add)
                nc.scalar.activation(out=scr[:], in_=py2[:], func=AF.Square,
                                     accum_out=l_sq_r[:, cg:cg + 1])

            # ---------- 5. x_vv canonical shard (needs AG1) ----------
            gv_all = gat.tile([P, NT_C, H], bf16, name="gv_all", tag="g8k", bufs=6)
            nc.gpsimd.dma_gather(
                gv_all[:], ag1_out[:], vidx_sb[:], num_idxs=4096, num_idxs_reg=4096,
                elem_size=H, single_packet=False, queue_num=3)
            for tb4 in range(NT_C // 4):
                sv4 = sgp.tile([P, 4, P], bf16, name="sv4")
                nc.sync.dma_start(out=sv4[:], in_=E_Svv[tb4])
                for tc4 in range(4):
                    t = tb4 * 4 + tc4
                    pv = sq("pv")
                    nc.tensor.matmul(out=pv[:], lhsT=sv4[:, tc4, :],
                                     rhs=gv_all[:, t, :], start=True, stop=True)
                    vnm = io.tile([P, H], bf16, name="vnm")
                    nc.scalar.copy(out=vnm[:], in_=pv[:])
                    nc.sync.dma_start(out=ag3_in[t * P:(t + 1) * P, H:2 * H],
                                      in_=vnm[:])

            if stage < 3:
                raise _StopBuild()
            # ---------- 6. global aggregation (needs AG2) ----------
            if stage == 32:
                ag2_loc = nc.dram_tensor("ag2_loc", [N_TOTAL, H], bf16)
                for tb in range(16):
                    cpt = io.tile([P, 16, H], bf16, name="cpt")
                    nc.sync.dma_start(
                        out=cpt[:],
                        in_=ag2_out[tb * 2048:(tb + 1) * 2048, :]
                        .rearrange("(a p) f -> p a f", p=P))
                    nc.sync.dma_start(
                        out=ag2_loc[tb * 2048:(tb + 1) * 2048, :]
                        .rearrange("(a p) f -> p a f", p=P),
                        in_=cpt[:])
                gsrc = ag2_loc
            else:
                gsrc = ag2_out
            aggT = big.tile([P, NC_SH], bf16)
            pagg = None
            for gq in range(NG_E):
                ge = gat.tile([P, 32, H], bf16, name="ge", tag="g8k", bufs=6)
                nc.gpsimd.dma_gather(
                    ge[:], gsrc[:], eidx_sb[:, gq * 256:(gq + 1) * 256],
                    num_idxs=4096, num_idxs_reg=4096, elem_size=H,
                    single_packet=False, queue_num=gq % 3)
                for j16 in range(2):
                    sg16 = sgp.tile([P, 16, P], bf16, name="sg16")
                    nc.sync.dma_start(
                        out=sg16[:],
                        in_=E_Sg[gq, :, j16 * 16:(j16 + 1) * 16, :])
                    for jj in range(16):
                        j = j16 * 16 + jj
                        ci = gq * 32 + j
                        t, k = ci // K_G, ci % K_G
                        if k == 0:
                            pagg = sq("pagg")
                        nc.tensor.matmul(out=pagg[:], lhsT=sg16[:, jj, :],
                                         rhs=ge[:, j, :],
                                         start=(k == 0), stop=(k == K_G - 1))
                        if k == K_G - 1:
                            anm = io.tile([P, H], bf16, name="anm")
                            nc.vector.tensor_copy(out=anm[:], in_=pagg[:])
                            nc.sync.dma_start(out=ag3_in[t * P:(t + 1) * P, 0:H],
                                              in_=anm[:])
                            paT = tp("paT")
                            nc.tensor.transpose(out=paT[:], in_=anm[:],
                                                identity=ident_b[:])
                            nc.scalar.copy(out=aggT[:, t * P:(t + 1) * P], in_=paT[:])
            if stage in (31, 32, 33, 34, 35):
                raise _StopBuild()
            nc.gpsimd.collective_compute(
                "AllGather", ALU.bypass, replica_groups=RG,
                ins=[ag3_in[:].opt()], outs=[ag3_out[:].opt()])

            if stage < 4:
                raise _StopBuild()
            # ---------- 7. global BN stats ----------
            g_sum_nr = stat.tile([P, NT_C // 4], f32)
            g_sq_nr = stat.tile([P, NT_C // 4], f32)
            g_sum_r = stat.tile([P, NT_C // 4], f32)
            g_sq_r = stat.tile([P, NT_C // 4], f32)
            for cg in range(NT_C // 4):
                sl = slice(cg * 512, (cg + 1) * 512)
                pg1 = wide("pg1")
                nc.tensor.matmul(out=pg1[:], lhsT=Wsb["gc_W"][:], rhs=aggT[:, sl],
                                 start=True, stop=True)
                nc.scalar.activation(out=scr[:], in_=pg1[:], func=AF.Identity,
                                     accum_out=g_sum_nr[:, cg:cg + 1])
                nc.scalar.activation(out=scr[:], in_=pg1[:], func=AF.Square,
                                     accum_out=g_sq_nr[:, cg:cg + 1])
                pg2 = wide("pg2")
                nc.tensor.matmul(out=pg2[:], lhsT=Wsb["gcr_W"][:], rhs=aggT[:, sl],
                                 start=True, stop=True)
                nc.vector.tensor_reduce(out=g_sum_r[:, cg:cg + 1], in_=pg2[:],
                                        axis=mybir.AxisListType.X, op=ALU.add)
                nc.scalar.activation(out=scr[:], in_=pg2[:], func=AF.Square,
                                     accum_out=g_sq_r[:, cg:cg + 1])

            # ---------- 8. stats AllReduce ----------
            ar_stage = stat.tile([P, 8], f32)
            for i, b in enumerate((l_sum_nr, l_sq_nr, l_sum_r, l_sq_r)):
                nc.vector.tensor_reduce(out=ar_stage[:, i:i + 1], in_=b[:],
                                        axis=mybir.AxisListType.X, op=ALU.add)
            for i, b in enumerate((g_sum_nr, g_sq_nr, g_sum_r, g_sq_r)):
                nc.vector.tensor_reduce(out=ar_stage[:, 4 + i:5 + i], in_=b[:],
                                        axis=mybir.AxisListType.X, op=ALU.add)
            nc.sync.dma_start(out=ar_in[:], in_=ar_stage[:])
            nc.gpsimd.collective_compute(
                "AllReduce", ALU.add, replica_groups=RG,
                ins=[ar_in[:].opt()], outs=[ar_out[:].opt()])
            stats_g = stat.tile([P, 8], f32)
            nc.sync.dma_start(out=stats_g[:], in_=ar_out[:])

            # ---------- 9. derive BN affine params, scaled weights ----------
            def bn_derive(sum_c, sq_c, n, gcol, bcol, name):
                mean = stat.tile([P, 1], f32, name=f"mean_{name}")
                nc.vector.tensor_scalar(out=mean[:], in0=stats_g[:, sum_c:sum_c + 1],
                                        scalar1=1.0 / n, scalar2=None, op0=ALU.mult)
                ex2 = stat.tile([P, 1], f32, name=f"ex2_{name}")
                nc.vector.tensor_scalar(out=ex2[:], in0=stats_g[:, sq_c:sq_c + 1],
                                        scalar1=1.0 / n, scalar2=None, op0=ALU.mult)
                var = stat.tile([P, 1], f32, name=f"var_{name}")
                nc.vector.tensor_tensor(out=var[:], in0=mean[:], in1=mean[:], op=ALU.mult)
                nc.vector.tensor_sub(out=var[:], in0=ex2[:], in1=var[:])
                std = stat.tile([P, 1], f32, name=f"std_{name}")
                nc.scalar.activation(out=std[:], in_=var[:], func=AF.Sqrt,
                                     bias=eps_t[:, :1])
                rstd = stat.tile([P, 1], f32, name=f"rstd_{name}")
                nc.vector.reciprocal(out=rstd[:], in_=std[:])
                sg = stat.tile([P, 1], f32, name=f"sg_{name}")
                nc.vector.tensor_tensor(out=sg[:], in0=rstd[:],
                                        in1=vecs[:, gcol:gcol + 1], op=ALU.mult)
                nb = stat.tile([P, 1], f32, name=f"nb_{name}")
                nc.vector.tensor_tensor(out=nb[:], in0=mean[:], in1=sg[:], op=ALU.mult)
                nc.vector.tensor_sub(out=nb[:], in0=vecs[:, bcol:bcol + 1], in1=nb[:])
                return sg, nb

            sg_lnr, nb_lnr = bn_derive(0, 1, F_, 3, 4, "lnr")
            sg_lr, nb_lr = bn_derive(2, 3, F_, 5, 6, "lr")
            sg_gnr, nb_gnr = bn_derive(4, 5, N_TOTAL, 7, 8, "gnr")
            sg_gr, nb_gr = bn_derive(6, 7, N_TOTAL, 9, 10, "gr")

            def scaled_w(wtname, sg, name):
                sc = io.tile([P, P], bf16, name=f"sc_{name}")
                nc.vector.tensor_scalar(out=sc[:], in0=Wsb[wtname][:], scalar1=sg[:, :1],
                                        scalar2=None, op0=ALU.mult)
                pw = tp(f"pw_{name}")
                nc.tensor.transpose(out=pw[:], in_=sc[:], identity=ident_b[:])
                w_s = const.tile([P, P], bf16, name=f"ws_{name}")
                nc.vector.tensor_copy(out=w_s[:], in_=pw[:])
                return w_s

            lcW_s = scaled_w("lc_WT", sg_lnr, "lc")
            gcW_s = scaled_w("gc_WT", sg_gnr, "gc")

            def diff_w(wt_r, wt_nr, sg_r, sg_nr, name):
                d1 = io.tile([P, P], f32, name=f"d1_{name}")
                nc.vector.tensor_scalar(out=d1[:], in0=Wsb[wt_r][:], scalar1=sg_r[:, :1],
                                        scalar2=None, op0=ALU.mult)
                d2 = io.tile([P, P], f32, name=f"d2_{name}")
                nc.vector.tensor_scalar(out=d2[:], in0=Wsb[wt_nr][:], scalar1=sg_nr[:, :1],
                                        scalar2=None, op0=ALU.mult)
                db = io.tile([P, P], bf16, name=f"db_{name}")
                nc.vector.tensor_sub(out=db[:], in0=d1[:], in1=d2[:])
                pw = tp(f"pwd_{name}")
                nc.tensor.transpose(out=pw[:], in_=db[:], identity=ident_b[:])
                w_d = const.tile([P, P], bf16, name=f"wd_{name}")
                nc.vector.tensor_copy(out=w_d[:], in_=pw[:])
                return w_d

            Wd_l = diff_w("lcr_WT", "lc_WT", sg_lr, sg_lnr, "l")
            Wd_g = diff_w("gcr_WT", "gc_WT", sg_gr, sg_gnr, "g")

            bias_tot = stat.tile([P, 1], f32)
            nc.vector.tensor_tensor(out=bias_tot[:], in0=vecs[:, 0:1], in1=vecs[:, 1:2],
                                    op=ALU.add)
            nc.vector.tensor_tensor(out=bias_tot[:], in0=bias_tot[:], in1=vecs[:, 2:3],
                                    op=ALU.add)
            nc.vector.tensor_tensor(out=bias_tot[:], in0=bias_tot[:], in1=nb_lnr[:],
                                    op=ALU.add)
            nc.vector.tensor_tensor(out=bias_tot[:], in0=bias_tot[:], in1=nb_gnr[:],
                                    op=ALU.add)
            bias_dlg = stat.tile([P, 1], f32)
            nc.vector.tensor_sub(out=bias_dlg[:], in0=nb_lr[:], in1=nb_lnr[:])
            tmp_dg = stat.tile([P, 1], f32)
            nc.vector.tensor_sub(out=tmp_dg[:], in0=nb_gr[:], in1=nb_gnr[:])
            nc.vector.tensor_tensor(out=bias_dlg[:], in0=bias_dlg[:], in1=tmp_dg[:],
                                    op=ALU.add)

            if stage < 5:
                raise _StopBuild()
            # ---------- 11. flat gathers + final assembly ----------
            for fg in range(8):
                gfa = gat.tile([P, 16, 2 * H], bf16, name="gfa", tag="g8k", bufs=6)
                nc.gpsimd.dma_gather(
                    gfa[:], ag3_out[:], fidx_sb[:, fg * 128:(fg + 1) * 128],
                    num_idxs=2048, num_idxs_reg=2048, elem_size=2 * H,
                    single_packet=False, queue_num=fg % 4)
                for cgl in range(4):
                    cg = fg * 4 + cgl
                    h4f = io.tile([P, 4, H], f32, name="h4f", bufs=2)
                    nc.sync.dma_start(out=h4f[:], in_=E_ht[cg])
                    b4f = sgp.tile([P, 4, P], bf16, name="b4f")
                    nc.sync.dma_start(out=b4f[:], in_=E_BT[cg])
                    gaT = io.tile([P, 512], bf16, name="gaT")
                    gvT = io.tile([P, 512], bf16, name="gvT")
                    rtG = io.tile([P, 512], bf16, name="rtG")
                    htG = io.tile([P, 512], bf16, name="htG")
                    for j in range(4):
                        b = cgl * 8 + j if False else (cgl * 4 + j)
                        hbf2 = io.tile([P, H], bf16, name="hbf2")
                        nc.vector.tensor_copy(out=hbf2[:], in_=h4f[:, j, :])
                        pr = sq("pr")
                        nc.tensor.matmul(out=pr[:], lhsT=hbf2[:], rhs=b4f[:, j, :],
                                         start=True, stop=True)
                        nc.vector.tensor_copy(out=rtG[:, j * P:(j + 1) * P], in_=pr[:])
                        ph = sq("ph")
                        nc.tensor.matmul(out=ph[:], lhsT=hbf2[:], rhs=ident_b[:],
                                         start=True, stop=True)
                        nc.scalar.copy(out=htG[:, j * P:(j + 1) * P], in_=ph[:])
                        pta = tp("pta")
                        nc.tensor.transpose(out=pta[:], in_=gfa[:, b, 0:H],
                                            identity=ident_b[:])
                        nc.vector.tensor_copy(out=gaT[:, j * P:(j + 1) * P], in_=pta[:])
                        ptv = tp("ptv")
                        nc.tensor.transpose(out=ptv[:], in_=gfa[:, b, H:2 * H],
                                            identity=ident_b[:])
                        nc.scalar.copy(out=gvT[:, j * P:(j + 1) * P], in_=ptv[:])
                    sl = slice(cg * 512, (cg + 1) * 512)
                    ptot = wide("ptot")
                    nc.tensor.matmul(out=ptot[:], lhsT=Wsb["skip_W"][:], rhs=htG[:],
                                     start=True, stop=False)
                    nc.tensor.matmul(out=ptot[:], lhsT=Wsb["kk_W"][:], rhs=rtG[:],
                                     start=False, stop=False)
                    nc.tensor.matmul(out=ptot[:], lhsT=lcW_s[:], rhs=ZT[:, sl],
                                     start=False, stop=False)
                    nc.tensor.matmul(out=ptot[:], lhsT=gcW_s[:], rhs=gaT[:],
                                     start=False, stop=False)
                    nc.tensor.matmul(out=ptot[:], lhsT=Wsb["vv_W"][:], rhs=gvT[:],
                                     start=False, stop=True)
                    tot_sb = io.tile([P, 512], f32, name="tot_sb")
                    nc.scalar.activation(out=tot_sb[:], in_=ptot[:], func=AF.Identity,
                                         bias=bias_tot[:, :1])
                    pdif = wide("pdif")
                    nc.tensor.matmul(out=pdif[:], lhsT=Wd_l[:], rhs=ZT[:, sl],
                                     start=True, stop=False)
                    nc.tensor.matmul(out=pdif[:], lhsT=Wd_g[:], rhs=gaT[:],
                                     start=False, stop=True)
                    dif_sb = io.tile([P, 512], f32, name="dif_sb")
                    nc.scalar.activation(out=dif_sb[:], in_=pdif[:], func=AF.Identity,
                                         bias=bias_dlg[:, :1])
                    out4 = io.tile([P, 4, H], f32, name="out4")
                    for j in range(4):
                        g = cg * 4 + j
                        po = sq("po")
                        nc.tensor.transpose(out=po[:], in_=tot_sb[:, j * P:(j + 1) * P],
                                            identity=ident_f[:])
                        pd = sq("pd")
                        nc.tensor.transpose(out=pd[:], in_=dif_sb[:, j * P:(j + 1) * P],
                                            identity=ident_f[:])
                        dm = io.tile([P, P], f32, name="dm")
                        nc.vector.tensor_scalar(out=dm[:], in0=pd[:],
                                                scalar1=isroot_sb[:, g:g + 1],
                                                scalar2=None, op0=ALU.mult)
                        sm = io.tile([P, P], f32, name="sm")
                        nc.vector.tensor_tensor(out=sm[:], in0=po[:], in1=dm[:],
                                                op=ALU.add)
                        nc.scalar.activation(out=out4[:, j, :], in_=sm[:], func=AF.Relu,
                                             scale=valid_sb[:, g:g + 1])
                    nc.sync.dma_start(
                        out=E_out[cg * 512:(cg + 1) * 512, :]
                        .rearrange("(a p) f -> p a f", p=P),
                        in_=out4[:])
        except _StopBuild:
            zt = io.tile([P, 512], f32, name="dummy_z")
            nc.vector.memset(zt[:], 0.0)
            for cgz in range(FC // 512):
                nc.sync.dma_start(
                    out=E_out[cgz * 512:(cgz + 1) * 512, :]
                    .rearrange("(a p) f -> p a f", p=P),
                    in_=zt[:].rearrange("p (a f) -> p a f", a=4))
        finally:
            for p_ in reversed(ctx_pools):
                p_.__exit__(None, None, None)

    nc.finalize()
    return nc


_NC_CACHE = {}
LAST_EXEC_NS = None


def kernel(**inputs) -> np.ndarray:
    from concourse.bass_utils import run_bass_kernel_spmd

    meta, cores = _host_prep(
        inputs["h_flat"], inputs["intra_ei"], inputs["valid"], inputs["node_ids"],
        inputs["edge_index"], inputs["sub_batch"], inputs["root_flat_idx"],
        inputs["is_root"])
    w = _host_weights(inputs)
    import os
    stage = int(os.environ.get("KSTAGE", "99"))
    key = (meta["K_G"], meta["NG_E"], stage)
    if key not in _NC_CACHE:
        _NC_CACHE[key] = _build_nc(key[0], key[1], stage=stage)
    nc = _NC_CACHE[key]

    in_maps = []
    for c in range(M):
        m = dict(cores[c])
        for k in ("skip_W", "vv_W", "kk_W", "lc_W", "lcr_W", "gc_W", "gcr_W"):
            m[k] = w[k]
        m["lc_WT"], m["lcr_WT"] = w["lc_WT"], w["lcr_WT"]
        m["gc_WT"], m["gcr_WT"] = w["gc_WT"], w["gcr_WT"]
        m["vecs"] = w["vecs"]
        in_maps.append(m)

    import os
    res = run_bass_kernel_spmd(nc, in_maps, list(range(M)))
    global LAST_EXEC_NS
    LAST_EXEC_NS = res.exec_time_ns
    out = np.concatenate([res.results[c]["out"] for c in range(M)], axis=0)
    return out.astype(np.float32)


# revision 34
# speedup vs baseline: 1.0734x; 1.0734x over previous
import sys

sys.path.insert(0, "/opt/trn_rl_repo")

import numpy as np
import ml_dtypes

BF16 = ml_dtypes.bfloat16

# Problem constants (nn_Arch7V2Layer: F=131072 flat nodes in 4096 subgraphs of 32,
# N=32768 canonical nodes, H=128, 524288 edges in each graph)
H = 128
P = 128
S_ = 4096
K_ = 32
F_ = S_ * K_
N_TOTAL = 32768
M = 8                    # cores
FC = F_ // M             # 16384 flat rows per core
NC_SH = N_TOTAL // M     # 4096 canonical rows per core
NT_C = NC_SH // P        # 32 canonical tiles per core
NT_F = FC // P           # 128 flat tiles per core
XS_T = N_TOTAL // P      # 256 canonical tiles (full range, x_sum partial)
BN_EPS = 1e-5


def _wrap_idx16(block):
    """dma_gather index layout: idx[p, s] = flat[s*16 + (p%16)], 128 partitions."""
    n = block.shape[0]
    assert n % 16 == 0
    m = block.reshape(n // 16, 16).T.astype(np.int16)   # [16, n/16]
    return np.tile(m, (8, 1))                           # [128, n/16]


def _host_prep(h_flat, intra_ei, valid, node_ids, edge_index, sub_batch,
               root_flat_idx, is_root):
    """Index-only preprocessing + sharding. Returns (meta, per-core input dicts)."""
    h_flat = np.asarray(h_flat, np.float32)
    intra_ei = np.asarray(intra_ei, np.int64)
    valid = np.asarray(valid)
    node_ids = np.asarray(node_ids, np.int64)
    edge_index = np.asarray(edge_index, np.int64)
    sub_batch = np.asarray(sub_batch, np.int64)
    root_flat_idx = np.asarray(root_flat_idx, np.int64)
    is_root = np.asarray(is_root)

    # ---- local graph degrees (self loops included) over the full flat graph
    ldeg = np.ones(F_, np.float64)
    np.add.at(ldeg, intra_ei[1], 1.0)
    ldinv = ldeg ** -0.5

    # ---- global graph degrees
    gdeg = np.ones(N_TOTAL, np.float64)
    np.add.at(gdeg, edge_index[1], 1.0)
    gdinv = gdeg ** -0.5

    # ---- global scatter-mean counts
    vmask = node_ids >= 0
    cids = np.where(vmask, node_ids, 0)
    cnt = np.zeros(N_TOTAL, np.float64)
    np.add.at(cnt, cids[vmask], 1.0)
    wxs_all = 1.0 / np.maximum(cnt, 1.0)

    root_ids = node_ids[root_flat_idx]          # [4096]
    rvalid = root_ids >= 0
    crids = np.where(rvalid, root_ids, 0)
    cntv = np.zeros(N_TOTAL, np.float64)
    np.add.at(cntv, crids[rvalid], 1.0)
    wvv_all = 1.0 / np.maximum(cntv, 1.0)

    # global edges + self loops, with weights
    ge_src = np.concatenate([edge_index[0], np.arange(N_TOTAL, dtype=np.int64)])
    ge_dst = np.concatenate([edge_index[1], np.arange(N_TOTAL, dtype=np.int64)])
    ge_w = gdinv[ge_src] * gdinv[ge_dst]

    root_of_flat = root_flat_idx[sub_batch]     # [F]

    # ---- uniform padding budgets (must be identical across cores)
    # K_G: global-edge chunks per canonical tile
    owner = ge_dst // NC_SH
    tile_g = ge_dst // P
    cnt_gt = np.bincount(tile_g, minlength=XS_T)
    K_G = int(np.ceil(cnt_gt.max() / P))
    # K_XS: x_sum rows per canonical tile per core
    maxr = 0
    for c in range(M):
        ids_c = cids[c * FC:(c + 1) * FC]
        vm_c = vmask[c * FC:(c + 1) * FC]
        bc = np.bincount(ids_c[vm_c] // P, minlength=XS_T)
        maxr = max(maxr, int(bc.max()))
    K_XS = int(np.ceil(maxr / P))
    assert K_XS == 1, f"K_XS={K_XS} unsupported"
    # K_VV: root rows per canonical tile
    bcv = np.bincount(crids[rvalid] // P, minlength=XS_T)
    K_VV = int(np.ceil(bcv.max() / P))
    assert K_VV == 1, f"K_VV={K_VV} unsupported"

    if K_G % 2 == 1:
        K_G += 1                                # keep 4096-idx gather blocks exact
    NG_E = (NT_C * K_G * P) // 4096             # dma_gather groups (32 chunks each)
    assert (NT_C * K_G * P) % 4096 == 0

    meta = dict(K_G=K_G, NG_E=NG_E)
    cores = []
    for c in range(M):
        lo = c * FC
        rows = slice(lo, lo + FC)
        h32 = np.ascontiguousarray(h_flat[rows])

        # ---- local block matrices A^T (adjacency incl. self loops) and B^T (root broadcast)
        es, ed = intra_ei[0], intra_ei[1]
        m = (ed >= lo) & (ed < lo + FC)
        assert ((es[m] >= lo) & (es[m] < lo + FC)).all(), "intra edge crosses shard"
        sl = (es[m] - lo).astype(np.int64)
        dl = (ed[m] - lo).astype(np.int64)
        AT = np.zeros((NT_F, P, P), np.float32)
        w_l = (ldinv[es[m]] * ldinv[ed[m]]).astype(np.float64)
        g_of = dl // P
        assert (sl // P == g_of).all()
        np.add.at(AT, (g_of, sl % P, dl % P), w_l)
        loc = np.arange(FC)
        AT[loc // P, loc % P, loc % P] += (ldinv[lo:lo + FC] ** 2)
        BT = np.zeros((NT_F, P, P), np.float32)
        rof = root_of_flat[rows] - lo
        assert ((rof >= 0) & (rof < FC)).all(), "root outside shard"
        assert (rof // P == loc // P).all()
        BT[loc // P, rof % P, loc % P] = 1.0

        isrootF = np.ascontiguousarray(
            is_root[rows].astype(np.float32).reshape(NT_F, P).T)
        validF = np.ascontiguousarray(
            valid[rows].astype(np.float32).reshape(NT_F, P).T)

        # ---- x_sum partial: sorted-by-id rows, padded per canonical tile
        ids_c = cids[rows]
        vm_c = vmask[rows]
        order = np.argsort(ids_c + (~vm_c) * (1 << 40), kind="stable")
        order = order[vm_c[order]]
        sids = ids_c[order]
        hperm = np.zeros((XS_T * K_XS * P, H), np.float32)
        Sxs = np.zeros((XS_T * K_XS, P, P), np.float32)
        tiles = sids // P
        starts = np.searchsorted(tiles, np.arange(XS_T))
        ends = np.searchsorted(tiles, np.arange(XS_T), side="right")
        for t in range(XS_T):
            n = ends[t] - starts[t]
            if n == 0:
                continue
            rsel = order[starts[t]:ends[t]]
            base = t * K_XS * P
            hperm[base:base + n] = h_flat[rows][rsel]
            Sxs[t * K_XS + (np.arange(n) // P), np.arange(n) % P,
                sids[starts[t]:ends[t]] % P] = wxs_all[sids[starts[t]:ends[t]]]

        # ---- global edges owned by this core, sorted by dst, padded per tile
        selg = owner == c
        gs = ge_src[selg]
        gd = ge_dst[selg]
        gw = ge_w[selg]
        og = np.argsort(gd, kind="stable")
        gs, gd, gw = gs[og], gd[og], gw[og]
        tl = (gd - c * NC_SH) // P
        st = np.searchsorted(tl, np.arange(NT_C))
        en = np.searchsorted(tl, np.arange(NT_C), side="right")
        esrc_pad = np.zeros(NT_C * K_G * P, np.int64)
        Sg = np.zeros((NT_C * K_G, P, P), np.float32)
        for t in range(NT_C):
            n = en[t] - st[t]
            assert n <= K_G * P
            base = t * K_G * P
            esrc_pad[base:base + n] = gs[st[t]:en[t]]
            Sg[t * K_G + (np.arange(n) // P), np.arange(n) % P,
               gd[st[t]:en[t]] % P] = gw[st[t]:en[t]]
        ep = esrc_pad
        esrc_phys = (ep // 512) * 512 + (ep % P) * 4 + (ep // P) % 4
        eidx = np.concatenate(
            [_wrap_idx16(esrc_phys[g * 4096:(g + 1) * 4096]) for g in range(NG_E)],
            axis=1)

        # ---- flat gathers (node_ids per row), root gathers
        fidx = np.concatenate(
            [_wrap_idx16(cids[rows][g * 2048:(g + 1) * 2048]) for g in range(8)],
            axis=1)
        rsel_roots = (root_flat_idx >= lo) & (root_flat_idx < lo + FC)
        r_glob = np.nonzero(rsel_roots)[0]
        assert len(r_glob) == S_ // M, f"roots per core {len(r_glob)}"
        my_root_ids = crids[r_glob]
        ridx = _wrap_idx16(my_root_ids)                       # [128, 32]
        rloc = _wrap_idx16((root_flat_idx[r_glob] - lo))      # [128, 32] int16

        # P matrices: roots of each output tile (4 per tile, in order)
        Pm = np.zeros((4, NT_F, P), np.float32)
        rl_flat = (root_flat_idx[r_glob] - lo)
        tg = rl_flat // P
        assert (np.bincount(tg, minlength=NT_F) == 4).all(), "need 4 roots/tile"
        kk_idx = np.zeros(NT_F, np.int64)
        for j, (t, rr) in enumerate(zip(tg, rl_flat % P)):
            Pm[kk_idx[t], t, rr] = 1.0
            kk_idx[t] += 1

        # ---- x_vv: roots landing in this core's canonical shard
        selv = rvalid & (crids // NC_SH == c)
        rv = np.nonzero(selv)[0]
        rvids = crids[rv]
        ov = np.argsort(rvids, kind="stable")
        rv, rvids = rv[ov], rvids[ov]
        tlv = (rvids - c * NC_SH) // P
        stv = np.searchsorted(tlv, np.arange(NT_C))
        env = np.searchsorted(tlv, np.arange(NT_C), side="right")
        vidx = np.zeros((NT_C * K_VV * P,), np.int32)
        Svv = np.zeros((NT_C * K_VV, P, P), np.float32)
        for t in range(NT_C):
            n = env[t] - stv[t]
            assert n <= K_VV * P
            base = t * K_VV * P
            vidx[base:base + n] = rv[stv[t]:env[t]]
            Svv[t * K_VV + (np.arange(n) // P), np.arange(n) % P,
                rvids[stv[t]:env[t]] % P] = wvv_all[rvids[stv[t]:env[t]]]
        vidx = _wrap_idx16(vidx)                              # [128, 256] int16

        def tile4(a, nb):
            return np.ascontiguousarray(
                a.reshape(nb, 4, P, a.shape[-1]).transpose(0, 2, 1, 3))

        cores.append(dict(
            h=h32,
            h_t=tile4(h32, NT_F // 4).astype(np.float32),
            hperm=hperm.astype(BF16),
            hperm_t=tile4(hperm, XS_T // 4).astype(BF16),
            AT=AT.astype(BF16), BT=BT.astype(BF16),
            AT_t=tile4(AT.reshape(-1, P), NT_F // 4).astype(BF16),
            BT_t=tile4(BT.reshape(-1, P), NT_F // 4).astype(BF16),
            Sxs=Sxs.astype(BF16), Sg=Sg.astype(BF16), Svv=Svv.astype(BF16),
            Sxs_t=tile4(Sxs.reshape(-1, P), XS_T // 4).astype(BF16),
            Sg_t=np.ascontiguousarray(
                Sg.reshape(NG_E, 32, P, P).transpose(0, 2, 1, 3)).astype(BF16),
            Svv_t=tile4(Svv.reshape(-1, P), NT_C // 4).astype(BF16),
            eidx=eidx.astype(np.int16), fidx=fidx.astype(np.int16),
            vidx=vidx.astype(np.int16), rloc=rloc.astype(np.int16),
            isrootF=isrootF, validF=validF,
        ))
    return meta, cores


def _host_weights(ins):
    """Per-core replicated weight/vector uploads."""
    w = {}
    for k in ("skip_W", "vv_W", "kk_W", "lc_W", "lcr_W", "gc_W", "gcr_W"):
        w[k] = np.asarray(ins[k], np.float32)
    for k in ("lc_W", "lcr_W", "gc_W", "gcr_W"):
        w[k + "T"] = np.ascontiguousarray(w[k].T)
    vec_names = ("skip_b", "kk_b", "vv_b", "lbn_g", "lbn_b", "lbnr_g", "lbnr_b",
                 "gbn_g", "gbn_b", "gbnr_g", "gbnr_b")
    w["vecs"] = np.stack([np.asarray(ins[k], np.float32) for k in vec_names],
                         axis=1)  # [128, 11]
    return w


class _StopBuild(Exception):
    pass


def _build_nc(K_G, NG_E, stage=99):
    from concourse import bass, bacc, mybir
    import concourse.tile as tile
    from concourse.masks import make_identity

    f32 = mybir.dt.float32
    bf16 = mybir.dt.bfloat16
    AF = mybir.ActivationFunctionType
    ALU = mybir.AluOpType

    nc = bacc.Bacc(None, num_swdge_queues=4)
    dp = nc.declare_dram_parameter
    E_h = dp("h", [FC, H], f32, isOutput=False)
    E_ht = dp("h_t", [NT_F // 4, P, 4, H], f32, isOutput=False)
    E_hperm = dp("hperm_t", [XS_T // 4, P, 4, H], bf16, isOutput=False)
    E_AT = dp("AT_t", [NT_F // 4, P, 4, P], bf16, isOutput=False)
    E_BT = dp("BT_t", [NT_F // 4, P, 4, P], bf16, isOutput=False)
    E_Sxs = dp("Sxs_t", [XS_T // 4, P, 4, P], bf16, isOutput=False)
    E_Sg = dp("Sg_t", [NG_E, P, 32, P], bf16, isOutput=False)
    E_Svv = dp("Svv_t", [NT_C // 4, P, 4, P], bf16, isOutput=False)
    E_eidx = dp("eidx", [P, NG_E * 256], mybir.dt.int16, isOutput=False)
    E_fidx = dp("fidx", [P, 8 * 128], mybir.dt.int16, isOutput=False)
    E_vidx = dp("vidx", [P, 256], mybir.dt.int16, isOutput=False)
    E_rloc = dp("rloc", [P, 32], mybir.dt.int16, isOutput=False)
    E_isroot = dp("isrootF", [P, NT_F], f32, isOutput=False)
    E_valid = dp("validF", [P, NT_F], f32, isOutput=False)
    WN = ("skip_W", "vv_W", "kk_W", "lc_W", "lcr_W", "gc_W", "gcr_W",
          "lc_WT", "lcr_WT", "gc_WT", "gcr_WT")
    E_W = {k: dp(k, [P, P], f32, isOutput=False) for k in WN}
    E_vecs = dp("vecs", [P, 11], f32, isOutput=False)
    E_out = dp("out", [FC, H], f32, isOutput=True)

    # collective bounce buffers
    ag1_in = nc.dram_tensor("ag1_in", [S_ // M, H], bf16)
    ag1_out = nc.dram_tensor("ag1_out", [S_, H], bf16, addr_space="Shared")
    rs_in = nc.dram_tensor("rs_in", [XS_T // 4, P, 4, H], bf16)
    rs_out = nc.dram_tensor("rs_out", [XS_T // 4 // M, P, 4, H], bf16)
    ag2_out = nc.dram_tensor("ag2_out", [N_TOTAL, H], bf16, addr_space="Shared")
    ag3_in = nc.dram_tensor("ag3_in", [NC_SH, 2 * H], bf16)
    ag3_out = nc.dram_tensor("ag3_out", [N_TOTAL, 2 * H], bf16, addr_space="Shared")
    ar_in = nc.dram_tensor("ar_in", [P, 8], f32)
    ar_out = nc.dram_tensor("ar_out", [P, 8], f32, addr_space="Shared")
    RG = [list(range(M))]

    with tile.TileContext(nc) as tc:
        ctx_pools = []

        def pool(name, bufs, space="SBUF"):
            p_ = tc.tile_pool(name=name, bufs=bufs, space=space)
            ctx_pools.append(p_)
            return p_.__enter__()

        try:
            const = pool("const", 1)
            io = pool("io", 2)
            ps = pool("ps", 1, "PSUM")
            big = pool("big", 1)
            stat = pool("stat", 1)
            gat = pool("gat", 2)
            sgp = pool("sgp", 2)

            def sq(name):
                return ps.tile([P, P], f32, name=name, tag="sq", bufs=4, space="PSUM")

            def wide(name):
                return ps.tile([P, 512], f32, name=name, tag="wide", bufs=2,
                               space="PSUM")

            def tp(name):
                return ps.tile([P, P], bf16, name=name, tag="tp", bufs=2, space="PSUM")

            # ---------- constants ----------
            ident_f = const.tile([P, P], f32)
            make_identity(nc, ident_f[:])
            eps_t = const.tile([P, 1], f32)
            nc.vector.memset(eps_t[:], BN_EPS)
            ident_b = const.tile([P, P], bf16)
            nc.vector.tensor_copy(out=ident_b[:], in_=ident_f[:])

            Wsb = {}
            for k in WN:
                t_f = io.tile([P, P], f32, name=f"wld_{k}")
                nc.sync.dma_start(out=t_f[:], in_=E_W[k][:])
                if k.endswith("T"):
                    Wsb[k] = const.tile([P, P], f32, name=f"w_{k}")
                    nc.vector.tensor_copy(out=Wsb[k][:], in_=t_f[:])
                else:
                    Wsb[k] = const.tile([P, P], bf16, name=f"w_{k}")
                    nc.vector.tensor_copy(out=Wsb[k][:], in_=t_f[:])
            vecs = const.tile([P, 11], f32)
            nc.sync.dma_start(out=vecs[:], in_=E_vecs[:])
            isroot_sb = const.tile([P, NT_F], f32)
            nc.sync.dma_start(out=isroot_sb[:], in_=E_isroot[:])
            valid_sb = const.tile([P, NT_F], f32)
            nc.sync.dma_start(out=valid_sb[:], in_=E_valid[:])
            eidx_sb = const.tile([P, NG_E * 256], mybir.dt.int16)
            nc.sync.dma_start(out=eidx_sb[:], in_=E_eidx[:])
            fidx_sb = const.tile([P, 8 * 128], mybir.dt.int16)
            nc.sync.dma_start(out=fidx_sb[:], in_=E_fidx[:])
            vidx_sb = const.tile([P, 256], mybir.dt.int16)
            nc.sync.dma_start(out=vidx_sb[:], in_=E_vidx[:])
            rloc_sb = const.tile([P, 32], mybir.dt.int16)
            nc.sync.dma_start(out=rloc_sb[:], in_=E_rloc[:])

            # ---------- 1. extract local roots, cast bf16, AG1 ----------
            ag1_stage = io.tile([P, 4, H], bf16, name="ag1_stage")
            rg = io.tile([P, 4, H], f32, name="rootg")
            nc.gpsimd.dma_gather(
                rg[:], E_h[:], rloc_sb[:], num_idxs=512, num_idxs_reg=512,
                elem_size=H, single_packet=False, queue_num=3)
            nc.vector.tensor_copy(out=ag1_stage[:], in_=rg[:])
            nc.sync.dma_start(
                out=ag1_in[:].rearrange("(a p) f -> p a f", p=P), in_=ag1_stage[:])
            nc.gpsimd.collective_compute(
                "AllGather", ALU.bypass, replica_groups=RG,
                ins=[ag1_in[:].opt()], outs=[ag1_out[:].opt()])

            # ---------- 2. x_sum partial via S matmuls, then RS ----------
            for tb in range(XS_T // 8):
                hp8 = io.tile([P, 2, 4, H], bf16, name="hp8")
                nc.scalar.dma_start(
                    out=hp8[:],
                    in_=E_hperm[tb * 2:(tb + 1) * 2].rearrange("b p a f -> p b a f"))
                sx8 = io.tile([P, 2, 4, P], bf16, name="sx8")
                nc.scalar.dma_start(
                    out=sx8[:],
                    in_=E_Sxs[tb * 2:(tb + 1) * 2].rearrange("b p a f -> p b a f"))
                st8 = io.tile([P, 2, 4, H], bf16, name="xs_st")
                for cc in range(8):
                    pxs = sq("pxs")
                    nc.tensor.matmul(out=pxs[:], lhsT=sx8[:, cc // 4, cc % 4, :],
                                     rhs=hp8[:, cc // 4, cc % 4, :],
                                     start=True, stop=True)
                    if cc % 2 == 0:
                        nc.vector.tensor_copy(out=st8[:, cc // 4, cc % 4, :], in_=pxs[:])
                    else:
                        nc.scalar.copy(out=st8[:, cc // 4, cc % 4, :], in_=pxs[:])
                nc.sync.dma_start(
                    out=rs_in[tb * 2:(tb + 1) * 2].rearrange("b p a f -> p b a f"),
                    in_=st8[:])
            nc.gpsimd.collective_compute(
                "ReduceScatter", ALU.add, replica_groups=RG,
                ins=[rs_in[:].opt()], outs=[rs_out[:].opt()])

            nc.gpsimd.collective_compute(
                "AllGather", ALU.bypass, replica_groups=RG,
                ins=[rs_out[:].opt()], outs=[ag2_out[:].opt()])

            # ---------- 3. local phase-1: Z^T (feat-major, bf16) ----------
            if stage < 2:
                raise _StopBuild()
            ZT = big.tile([P, FC], bf16)
            for gb in range(NT_F // 4):
                h4 = io.tile([P, 4, H], f32, name="h4")
                nc.sync.dma_start(out=h4[:], in_=E_ht[gb])
                a4 = io.tile([P, 4, P], bf16, name="a4")
                nc.sync.dma_start(out=a4[:], in_=E_AT[gb])
                for cc in range(4):
                    g = gb * 4 + cc
                    hbf = io.tile([P, H], bf16, name="hbf")
                    nc.vector.tensor_copy(out=hbf[:], in_=h4[:, cc, :])
                    pz = sq("pz")
                    nc.tensor.matmul(out=pz[:], lhsT=hbf[:], rhs=a4[:, cc, :],
                                     start=True, stop=True)
                    sl = slice(g * P, (g + 1) * P)
                    nc.vector.tensor_copy(out=ZT[:, sl], in_=pz[:])

            # ---------- 4. local BN stats (pre-AR partial sums) ----------
            l_sum_nr = stat.tile([P, NT_F // 4], f32)
            l_sq_nr = stat.tile([P, NT_F // 4], f32)
            l_sum_r = stat.tile([P, NT_F // 4], f32)
            l_sq_r = stat.tile([P, NT_F // 4], f32)
            scr = io.tile([P, 512], f32, name="scr")
            for cg in range(NT_F // 4):
                sl = slice(cg * 512, (cg + 1) * 512)
                py = wide("py")
                nc.tensor.matmul(out=py[:], lhsT=Wsb["lc_W"][:], rhs=ZT[:, sl],
                                 start=True, stop=True)
                nc.scalar.activation(out=scr[:], in_=py[:], func=AF.Identity,
                                     accum_out=l_sum_nr[:, cg:cg + 1])
                nc.scalar.activation(out=scr[:], in_=py[:], func=AF.Square,
                                     accum_out=l_sq_nr[:, cg:cg + 1])
                py2 = wide("py2")
                nc.tensor.matmul(out=py2[:], lhsT=Wsb["lcr_W"][:], rhs=ZT[:, sl],
                                 start=True, stop=True)
                nc.vector.tensor_reduce(out=l_sum_r[:, cg:cg + 1], in_=py2[:],
                                        axis=mybir.AxisListType.X, op=ALU.add)
                nc.scalar.activation(out=scr[:], in_=py2[:], func=AF.Square,
                                     accum_out=l_sq_r[:, cg:cg + 1])

            # ---------- 5. x_vv canonical shard (needs AG1) ----------
            gv_all = gat.tile([P, NT_C, H], bf16, name="gv_all", tag="g8k", bufs=5)
            nc.gpsimd.dma_gather(
                gv_all[:], ag1_out[:], vidx_sb[:], num_idxs=4096, num_idxs_reg=4096,
                elem_size=H, single_packet=False, queue_num=3)
            for tb4 in range(NT_C // 4):
                sv4 = sgp.tile([P, 4, P], bf16, name="sv4")
                nc.sync.dma_start(out=sv4[:], in_=E_Svv[tb4])
                for tc4 in range(4):
                    t = tb4 * 4 + tc4
                    pv = sq("pv")
                    nc.tensor.matmul(out=pv[:], lhsT=sv4[:, tc4, :],
                                     rhs=gv_all[:, t, :], start=True, stop=True)
                    vnm = io.tile([P, H], bf16, name="vnm")
                    nc.scalar.copy(out=vnm[:], in_=pv[:])
                    nc.sync.dma_start(out=ag3_in[t * P:(t + 1) * P, H:2 * H],
                                      in_=vnm[:])

            if stage < 3:
                raise _StopBuild()
            # ---------- 6. global aggregation (needs AG2) ----------
            if stage == 32:
                ag2_loc = nc.dram_tensor("ag2_loc", [N_TOTAL, H], bf16)
                for tb in range(16):
                    cpt = io.tile([P, 16, H], bf16, name="cpt")
                    nc.sync.dma_start(
                        out=cpt[:],
                        in_=ag2_out[tb * 2048:(tb + 1) * 2048, :]
                        .rearrange("(a p) f -> p a f", p=P))
                    nc.sync.dma_start(
                        out=ag2_loc[tb * 2048:(tb + 1) * 2048, :]
                        .rearrange("(a p) f -> p a f", p=P),
                        in_=cpt[:])
                gsrc = ag2_loc
            else:
                gsrc = ag2_out
            aggT = big.tile([P, NC_SH], bf16)
            pagg = None
            for gq in range(NG_E):
                ge = gat.tile([P, 32, H], bf16, name="ge", tag="g8k", bufs=5)
                nc.gpsimd.dma_gather(
                    ge[:], gsrc[:], eidx_sb[:, gq * 256:(gq + 1) * 256],
                    num_idxs=4096, num_idxs_reg=4096, elem_size=H,
                    single_packet=False, queue_num=gq % 3)
                for j16 in range(2):
                    sg16 = sgp.tile([P, 16, P], bf16, name="sg16")
                    nc.sync.dma_start(
                        out=sg16[:],
                        in_=E_Sg[gq, :, j16 * 16:(j16 + 1) * 16, :])
                    for jj in range(16):
                        j = j16 * 16 + jj
                        ci = gq * 32 + j
                        t, k = ci // K_G, ci % K_G
                        if k == 0:
                            pagg = sq("pagg")
                        nc.tensor.matmul(out=pagg[:], lhsT=sg16[:, jj, :],
                                         rhs=ge[:, j, :],
                                         start=(k == 0), stop=(k == K_G - 1))
                        if k == K_G - 1:
                            anm = io.tile([P, H], bf16, name="anm")
                            nc.vector.tensor_copy(out=anm[:], in_=pagg[:])
                            nc.sync.dma_start(out=ag3_in[t * P:(t + 1) * P, 0:H],
                                              in_=anm[:])
                            paT = tp("paT")
                            nc.tensor.transpose(out=paT[:], in_=anm[:],
                                                identity=ident_b[:])
                            nc.scalar.copy(out=aggT[:, t * P:(t + 1) * P], in_=paT[:])
            if stage in (31, 32, 33, 34, 35):
                raise _StopBuild()
            nc.gpsimd.collective_compute(
                "AllGather", ALU.bypass, replica_groups=RG,
                ins=[ag3_in[:].opt()], outs=[ag3_out[:].opt()])

            if stage < 4:
                raise _StopBuild()
            # ---------- 7. global BN stats ----------
            g_sum_nr = stat.tile([P, NT_C // 4], f32)
            g_sq_nr = stat.tile([P, NT_C // 4], f32)
            g_sum_r = stat.tile([P, NT_C // 4], f32)
            g_sq_r = stat.tile([P, NT_C // 4], f32)
            for cg in range(NT_C // 4):
                sl = slice(cg * 512, (cg + 1) * 512)
                pg1 = wide("pg1")
                nc.tensor.matmul(out=pg1[:], lhsT=Wsb["gc_W"][:], rhs=aggT[:, sl],
                                 start=True, stop=True)
                nc.scalar.activation(out=scr[:], in_=pg1[:], func=AF.Identity,
                                     accum_out=g_sum_nr[:, cg:cg + 1])
                nc.scalar.activation(out=scr[:], in_=pg1[:], func=AF.Square,
                                     accum_out=g_sq_nr[:, cg:cg + 1])
                pg2 = wide("pg2")
                nc.tensor.matmul(out=pg2[:], lhsT=Wsb["gcr_W"][:], rhs=aggT[:, sl],
                                 start=True, stop=True)
                nc.vector.tensor_reduce(out=g_sum_r[:, cg:cg + 1], in_=pg2[:],
                                        axis=mybir.AxisListType.X, op=ALU.add)
                nc.scalar.activation(out=scr[:], in_=pg2[:], func=AF.Square,
                                     accum_out=g_sq_r[:, cg:cg + 1])

            # ---------- 8. stats AllReduce ----------
            ar_stage = stat.tile([P, 8], f32)
            for i, b in enumerate((l_sum_nr, l_sq_nr, l_sum_r, l_sq_r)):
                nc.vector.tensor_reduce(out=ar_stage[:, i:i + 1], in_=b[:],
                                        axis=mybir.AxisListType.X, op=ALU.add)
            for i, b in enumerate((g_sum_nr, g_sq_nr, g_sum_r, g_sq_r)):
                nc.vector.tensor_reduce(out=ar_stage[:, 4 + i:5 + i], in_=b[:],
                                        axis=mybir.AxisListType.X, op=ALU.add)
            nc.sync.dma_start(out=ar_in[:], in_=ar_stage[:])
            nc.gpsimd.collective_compute(
                "AllReduce", ALU.add, replica_groups=RG,
                ins=[ar_in[:].opt()], outs=[ar_out[:].opt()])
            stats_g = stat.tile([P, 8], f32)
            nc.sync.dma_start(out=stats_g[:], in_=ar_out[:])

            # ---------- 9. derive BN affine params, scaled weights ----------
            def bn_derive(sum_c, sq_c, n, gcol, bcol, name):
                mean = stat.tile([P, 1], f32, name=f"mean_{name}")
                nc.vector.tensor_scalar(out=mean[:], in0=stats_g[:, sum_c:sum_c + 1],
                                        scalar1=1.0 / n, scalar2=None, op0=ALU.mult)
                ex2 = stat.tile([P, 1], f32, name=f"ex2_{name}")
                nc.vector.tensor_scalar(out=ex2[:], in0=stats_g[:, sq_c:sq_c + 1],
                                        scalar1=1.0 / n, scalar2=None, op0=ALU.mult)
                var = stat.tile([P, 1], f32, name=f"var_{name}")
                nc.vector.tensor_tensor(out=var[:], in0=mean[:], in1=mean[:], op=ALU.mult)
                nc.vector.tensor_sub(out=var[:], in0=ex2[:], in1=var[:])
                std = stat.tile([P, 1], f32, name=f"std_{name}")
                nc.scalar.activation(out=std[:], in_=var[:], func=AF.Sqrt,
                                     bias=eps_t[:, :1])
                rstd = stat.tile([P, 1], f32, name=f"rstd_{name}")
                nc.vector.reciprocal(out=rstd[:], in_=std[:])
                sg = stat.tile([P, 1], f32, name=f"sg_{name}")
                nc.vector.tensor_tensor(out=sg[:], in0=rstd[:],
                                        in1=vecs[:, gcol:gcol + 1], op=ALU.mult)
                nb = stat.tile([P, 1], f32, name=f"nb_{name}")
                nc.vector.tensor_tensor(out=nb[:], in0=mean[:], in1=sg[:], op=ALU.mult)
                nc.vector.tensor_sub(out=nb[:], in0=vecs[:, bcol:bcol + 1], in1=nb[:])
                return sg, nb

            sg_lnr, nb_lnr = bn_derive(0, 1, F_, 3, 4, "lnr")
            sg_lr, nb_lr = bn_derive(2, 3, F_, 5, 6, "lr")
            sg_gnr, nb_gnr = bn_derive(4, 5, N_TOTAL, 7, 8, "gnr")
            sg_gr, nb_gr = bn_derive(6, 7, N_TOTAL, 9, 10, "gr")

            def scaled_w(wtname, sg, name):
                sc = io.tile([P, P], bf16, name=f"sc_{name}")
                nc.vector.tensor_scalar(out=sc[:], in0=Wsb[wtname][:], scalar1=sg[:, :1],
                                        scalar2=None, op0=ALU.mult)
                pw = tp(f"pw_{name}")
                nc.tensor.transpose(out=pw[:], in_=sc[:], identity=ident_b[:])
                w_s = const.tile([P, P], bf16, name=f"ws_{name}")
                nc.vector.tensor_copy(out=w_s[:], in_=pw[:])
                return w_s

            lcW_s = scaled_w("lc_WT", sg_lnr, "lc")
            gcW_s = scaled_w("gc_WT", sg_gnr, "gc")

            def diff_w(wt_r, wt_nr, sg_r, sg_nr, name):
                d1 = io.tile([P, P], f32, name=f"d1_{name}")
                nc.vector.tensor_scalar(out=d1[:], in0=Wsb[wt_r][:], scalar1=sg_r[:, :1],
                                        scalar2=None, op0=ALU.mult)
                d2 = io.tile([P, P], f32, name=f"d2_{name}")
                nc.vector.tensor_scalar(out=d2[:], in0=Wsb[wt_nr][:], scalar1=sg_nr[:, :1],
                                        scalar2=None, op0=ALU.mult)
                db = io.tile([P, P], bf16, name=f"db_{name}")
                nc.vector.tensor_sub(out=db[:], in0=d1[:], in1=d2[:])
                pw = tp(f"pwd_{name}")
                nc.tensor.transpose(out=pw[:], in_=db[:], identity=ident_b[:])
                w_d = const.tile([P, P], bf16, name=f"wd_{name}")
                nc.vector.tensor_copy(out=w_d[:], in_=pw[:])
                return w_d

            Wd_l = diff_w("lcr_WT", "lc_WT", sg_lr, sg_lnr, "l")
            Wd_g = diff_w("gcr_WT", "gc_WT", sg_gr, sg_gnr, "g")

            bias_tot = stat.tile([P, 1], f32)
            nc.vector.tensor_tensor(out=bias_tot[:], in0=vecs[:, 0:1], in1=vecs[:, 1:2],
                                    op=ALU.add)
            nc.vector.tensor_tensor(out=bias_tot[:], in0=bias_tot[:], in1=vecs[:, 2:3],
                                    op=ALU.add)
            nc.vector.tensor_tensor(out=bias_tot[:], in0=bias_tot[:], in1=nb_lnr[:],
                                    op=ALU.add)
            nc.vector.tensor_tensor(out=bias_tot[:], in0=bias_tot[:], in1=nb_gnr[:],
                                    op=ALU.add)
            bias_dlg = stat.tile([P, 1], f32)
            nc.vector.tensor_sub(out=bias_dlg[:], in0=nb_lr[:], in1=nb_lnr[:])
            tmp_dg = stat.tile([P, 1], f32)
            nc.vector.tensor_sub(out=tmp_dg[:], in0=nb_gr[:], in1=nb_gnr[:])
            nc.vector.tensor_tensor(out=bias_dlg[:], in0=bias_dlg[:], in1=tmp_dg[:],
                                    op=ALU.add)

            if stage < 5:
                raise _StopBuild()
            # ---------- 11. flat gathers + final assembly ----------
            for fg in range(8):
                gfa = gat.tile([P, 16, 2 * H], bf16, name="gfa", tag="g8k", bufs=5)
                nc.gpsimd.dma_gather(
                    gfa[:], ag3_out[:], fidx_sb[:, fg * 128:(fg + 1) * 128],
                    num_idxs=2048, num_idxs_reg=2048, elem_size=2 * H,
                    single_packet=False, queue_num=fg % 4)
                for cgl in range(4):
                    cg = fg * 4 + cgl
                    h4f = io.tile([P, 4, H], f32, name="h4f", bufs=2)
                    nc.sync.dma_start(out=h4f[:], in_=E_ht[cg])
                    b4f = sgp.tile([P, 4, P], bf16, name="b4f")
                    nc.sync.dma_start(out=b4f[:], in_=E_BT[cg])
                    gaT = io.tile([P, 512], bf16, name="gaT")
                    gvT = io.tile([P, 512], bf16, name="gvT")
                    rtG = io.tile([P, 512], bf16, name="rtG")
                    htG = io.tile([P, 512], bf16, name="htG")
                    for j in range(4):
                        b = cgl * 8 + j if False else (cgl * 4 + j)
                        hbf2 = io.tile([P, H], bf16, name="hbf2")
                        nc.vector.tensor_copy(out=hbf2[:], in_=h4f[:, j, :])
                        pr = sq("pr")
                        nc.tensor.matmul(out=pr[:], lhsT=hbf2[:], rhs=b4f[:, j, :],
                                         start=True, stop=True)
                        nc.vector.tensor_copy(out=rtG[:, j * P:(j + 1) * P], in_=pr[:])
                        ph = sq("ph")
                        nc.tensor.matmul(out=ph[:], lhsT=hbf2[:], rhs=ident_b[:],
                                         start=True, stop=True)
                        nc.scalar.copy(out=htG[:, j * P:(j + 1) * P], in_=ph[:])
                        pta = tp("pta")
                        nc.tensor.transpose(out=pta[:], in_=gfa[:, b, 0:H],
                                            identity=ident_b[:])
                        nc.vector.tensor_copy(out=gaT[:, j * P:(j + 1) * P], in_=pta[:])
                        ptv = tp("ptv")
                        nc.tensor.transpose(out=ptv[:], in_=gfa[:, b, H:2 * H],
                                            identity=ident_b[:])
                        nc.scalar.copy(out=gvT[:, j * P:(j + 1) * P], in_=ptv[:])
                    sl = slice(cg * 512, (cg + 1) * 512)
                    ptot = wide("ptot")
                    nc.tensor.matmul(out=ptot[:], lhsT=Wsb["skip_W"][:], rhs=htG[:],
                                     start=True, stop=False)
                    nc.tensor.matmul(out=ptot[:], lhsT=Wsb["kk_W"][:], rhs=rtG[:],
                                     start=False, stop=False)
                    nc.tensor.matmul(out=ptot[:], lhsT=lcW_s[:], rhs=ZT[:, sl],
                                     start=False, stop=False)
                    nc.tensor.matmul(out=ptot[:], lhsT=gcW_s[:], rhs=gaT[:],
                                     start=False, stop=False)
                    nc.tensor.matmul(out=ptot[:], lhsT=Wsb["vv_W"][:], rhs=gvT[:],
                                     start=False, stop=True)
                    tot_sb = io.tile([P, 512], f32, name="tot_sb")
                    nc.scalar.activation(out=tot_sb[:], in_=ptot[:], func=AF.Identity,
                                         bias=bias_tot[:, :1])
                    pdif = wide("pdif")
                    nc.tensor.matmul(out=pdif[:], lhsT=Wd_l[:], rhs=ZT[:, sl],
                                     start=True, stop=False)
                    nc.tensor.matmul(out=pdif[:], lhsT=Wd_g[:], rhs=gaT[:],
                                     start=False, stop=True)
                    dif_sb = io.tile([P, 512], f32, name="dif_sb")
                    nc.scalar.activation(out=dif_sb[:], in_=pdif[:], func=AF.Identity,
                                         bias=bias_dlg[:, :1])
                    out4 = io.tile([P, 4, H], f32, name="out4")
                    for j in range(4):
                        g = cg * 4 + j
                        po = sq("po")
                        nc.tensor.transpose(out=po[:], in_=tot_sb[:, j * P:(j + 1) * P],
                                            identity=ident_f[:])
                        pd = sq("pd")
                        nc.tensor.transpose(out=pd[:], in_=dif_sb[:, j * P:(j + 1) * P],
                                            identity=ident_f[:])
                        dm = io.tile([P, P], f32, name="dm")
                        nc.vector.tensor_scalar(out=dm[:], in0=pd[:],
                                                scalar1=isroot_sb[:, g:g + 1],
                                                scalar2=None, op0=ALU.mult)
                        sm = io.tile([P, P], f32, name="sm")
                        nc.vector.tensor_tensor(out=sm[:], in0=po[:], in1=dm[:],
                                                op=ALU.add)
                        nc.scalar.activation(out=out4[:, j, :], in_=sm[:], func=AF.Relu,
                                             scale=valid_sb[:, g:g + 1])
                    nc.sync.dma_start(
                        out=E_out[cg * 512:(cg + 1) * 512, :]
                        .rearrange("(a p) f -> p a f", p=P),
                        in_=out4[:])
        except _StopBuild:
            zt = io.tile([P, 512], f32, name="dummy_z")
            nc.vector.memset(zt[:], 0.0)
            for cgz in range(FC // 512):
                nc.sync.dma_start(
                    out=E_out[cgz * 512:(cgz + 1) * 512, :]
                    .rearrange("(a p) f -> p a f", p=P),
                    in_=zt[:].rearrange("p (a f) -> p a f", a=4))
        finally:
            for p_ in reversed(ctx_pools):
                p_.__exit__(None, None, None)

    nc.finalize()
    return nc


_NC_CACHE = {}
LAST_EXEC_NS = None


def kernel(**inputs) -> np.ndarray:
    from concourse.bass_utils import run_bass_kernel_spmd

    meta, cores = _host_prep(
        inputs["h_flat"], inputs["intra_ei"], inputs["valid"], inputs["node_ids"],
        inputs["edge_index"], inputs["sub_batch"], inputs["root_flat_idx"],
        inputs["is_root"])
    w = _host_weights(inputs)
    import os
    stage = int(os.environ.get("KSTAGE", "99"))
    key = (meta["K_G"], meta["NG_E"], stage)
    if key not in _NC_CACHE:
        _NC_CACHE[key] = _build_nc(key[0], key[1], stage=stage)
    nc = _NC_CACHE[key]

    in_maps = []
    for c in range(M):
        m = dict(cores[c])
        for k in ("skip_W", "vv_W", "kk_W", "lc_W", "lcr_W", "gc_W", "gcr_W"):
            m[k] = w[k]
        m["lc_WT"], m["lcr_WT"] = w["lc_WT"], w["lcr_WT"]
        m["gc_WT"], m["gcr_WT"] = w["gc_WT"], w["gcr_WT"]
        m["vecs"] = w["vecs"]
        in_maps.append(m)

    import os
    res = run_bass_kernel_spmd(nc, in_maps, list(range(M)))
    global LAST_EXEC_NS
    LAST_EXEC_NS = res.exec_time_ns
    out = np.concatenate([res.results[c]["out"] for c in range(M)], axis=0)
    return out.astype(np.float32)
